# revision 35
# baseline (speedup 1.0000x reference)
"""Trainium2 Bass kernel for nn_CrossAttention (dense_transformer).

Reference computation (per batch b, per stream s in {1,2}):
    q_s   = heads(x_s)                      # [H, N, D] slices of x_s
    kv_s  = x_s @ Wkv_s -> k_s, v_s         # [N, C] each
    gate_s= sigmoid(relu(x_s @ w1 + b1) @ w2 + b2)
    ctx_s = softmax_d( scale * k_s^T @ (v_s * gate_s) )   # [H, D, D], softmax over d
    o_1   = q_1 @ ctx_2 ; o_2 = q_2 @ ctx_1  (cross)

Sharding: 8 cores = (stream s, batch b) pairs.  Core (s, b) projects
x_s[b] (kv + gate + ctx_s[b]) and then computes the OTHER stream's
output o_{1-s}[b] = q_{1-s}[b] @ softmax(ctx_s[b]).  No cross-core
communication; host concatenates outputs.

v2: host pre-transposes and fp16-casts x (so no on-chip transposes),
fp16 matmul operands everywhere (fp32 PSUM accumulate), fused
block-pipelined gate1 -> gate2/kv/vg -> ctx (per-chunk one-shot PSUM
partials DVE-accumulated in SBUF), pipelined softmax, then phase B
with spair-stationary wide matmuls producing o transposed (host
un-transposes).  xqT is fully prefetched during phase A so phase B's
DMA is writes-only.
"""

import numpy as np
from contextlib import ExitStack

N = 4096
C = 1024
H = 16
D = 64
SCALE = D ** (-0.5)
NBLK = 4            # n-blocks of 1024 rows
BN = N // NBLK      # 1024 rows per block
BCH = BN // 128     # 8 chunks of 128 rows per block

_CACHE = {}


def _build_program(with_bias):
    """Build the SPMD Bass program (same for all 8 cores)."""
    import concourse.bass as bass
    import concourse.bacc as bacc
    import concourse.tile as tile
    import concourse.mybir as mybir

    F32 = mybir.dt.float32
    F16 = mybir.dt.float16
    AF = mybir.ActivationFunctionType

    nc = bacc.Bacc("TRN2", target_bir_lowering=False, debug=False, num_devices=8)

    xpT = nc.dram_tensor("xpT", [C, N], F16, kind="ExternalInput").ap()
    xqT = nc.dram_tensor("xqT", [C, N], F16, kind="ExternalInput").ap()
    wkv = nc.dram_tensor("wkv", [C, 2 * C], F16, kind="ExternalInput").ap()
    w1 = nc.dram_tensor("w1", [C, C], F16, kind="ExternalInput").ap()
    w2 = nc.dram_tensor("w2", [C, C], F16, kind="ExternalInput").ap()
    b1 = nc.dram_tensor("b1", [C], F32, kind="ExternalInput").ap()
    b2 = nc.dram_tensor("b2", [C], F16, kind="ExternalInput").ap()
    identh = nc.dram_tensor("identh", [128, 128], F16, kind="ExternalInput").ap()
    maskh = nc.dram_tensor("maskh", [128, 128], F16, kind="ExternalInput").ap()
    # output is produced TRANSPOSED ([channel, n]); host un-transposes.
    o = nc.dram_tensor("o", [C, N], F16, kind="ExternalOutput").ap()

    with tile.TileContext(nc) as tc, ExitStack() as ctx:
        # ---------- persistent pools ----------
        cpool = ctx.enter_context(tc.tile_pool(name="consts", bufs=1))
        b1_sb = cpool.tile([128, 8], F32, name="b1_sb")  # b1_sb[p, m] = b1[m*128+p]
        identh_sb = cpool.tile([128, 128], F16, name="identh_sb")
        maskh_sb = cpool.tile([128, 128], F16, name="maskh_sb")
        if with_bias:
            ones_r = cpool.tile([1, 128], F16, name="ones_r")
            nc.vector.memset(ones_r, 1.0)
            b2_r = cpool.tile([1, C], F16, name="b2_r")
            nc.sync.dma_start(b2_r, b2.rearrange("(one f) -> one f", one=1))

        spool = ctx.enter_context(tc.tile_pool(name="spairs", bufs=1))
        spairs = [spool.tile([128, 128], F16, name=f"spair{j}") for j in range(8)]
        # preload the ScalarE Exp table now so the softmax-critical-path
        # activation doesn't pay the ~1.5us ACT_TABLE_LOAD later.
        expwarm = cpool.tile([1, 1], F32, name="expwarm")
        expwarm2 = cpool.tile([1, 1], F32, name="expwarm2")
        nc.vector.memset(expwarm, 0.0)
        nc.scalar.activation(expwarm2, expwarm, AF.Exp)

        # weights: w1 first (gate1 is the critical path at startup), split in
        # m-quarters so gate1 m=0 can start before the whole matrix lands.
        wpool = ctx.enter_context(tc.tile_pool(name="weights", bufs=1))
        w1r = w1.rearrange("(k p) m -> p k m", p=128)
        w1_sb = [wpool.tile([128, 8, C // 4], F16, name=f"w1_sb{qq}")
                 for qq in range(4)]
        nc.sync.dma_start(w1_sb[0], w1r[:, :, 0:C // 4])

        # ctx^T accumulator in SBUF, [e(2-head pair), d(2-head pair)] per pair j
        # at cols j*128; per-chunk partials land in a PSUM tile and are
        # DVE-accumulated here (one-shot matmul groups, like the baseline).
        acc_pool = ctx.enter_context(tc.tile_pool(name="ctxacc", bufs=1))
        ctx_acc = acc_pool.tile([128, 1024], F32, name="ctx_acc")
        nc.vector.memset(ctx_acc, 0.0)

        # xqT prefetch pool (phase B input), fully resident by end of phase A
        # so phase B's DMA is writes-only (reads+writes together oversubscribe
        # the 358 GB/s DMA during the short phase B window).
        xqt_pool = ctx.enter_context(tc.tile_pool(name="xqt", bufs=4))
        xqt_tiles = {}

        def load_xqt(blk):
            t = xqt_pool.tile([128, 8, BN], F16, name="xqt", tag="xqt")
            nc.sync.dma_start(
                t,
                xqT.rearrange("(k p) n -> p k n", p=128)[
                    :, :, blk * BN:(blk + 1) * BN
                ],
            )
            xqt_tiles[blk] = t

        # =========================================================
        # Phase A: gate MLP + kv projection + ctx accumulation,
        # fused per n-block of 1024 rows.
        # =========================================================
        with ExitStack() as pa:
            # xpT arrives in segments (block 0: four 256-col quarters so the
            # first gate1 group starts ~3us in; later blocks: 512-col halves).
            xpt_pool = pa.enter_context(tc.tile_pool(name="xpt", bufs=2))
            xpt_tiles = {}
            xpTr = xpT.rearrange("(k p) n -> p k n", p=128)

            def load_xpt_seg(pool, tag, blk, lo, ncols):
                t = pool.tile([128, 8, ncols], F16, name=tag, tag=tag)
                nc.sync.dma_start(
                    t, xpTr[:, :, blk * BN + lo:blk * BN + lo + ncols]
                )
                xpt_tiles.setdefault(blk, []).append((t, lo, ncols))

            ht_pool = pa.enter_context(tc.tile_pool(name="ht", bufs=1))
            g_pool = pa.enter_context(tc.tile_pool(name="g", bufs=2))
            k_pool = pa.enter_context(tc.tile_pool(name="k", bufs=2))
            vg_pool = pa.enter_context(tc.tile_pool(name="vg", bufs=2))
            g1ps_pool = pa.enter_context(
                tc.tile_pool(name="g1ps", bufs=2, space="PSUM")
            )
            g2ps_pool = pa.enter_context(
                tc.tile_pool(name="g2ps", bufs=2, space="PSUM")
            )
            kvps_pool = pa.enter_context(
                tc.tile_pool(name="kvps", bufs=2, space="PSUM")
            )
            ctps_pool = pa.enter_context(
                tc.tile_pool(name="ctps", bufs=1, space="PSUM")
            )
            # block-0 quarter tiles live only until block 0 is consumed
            xpt0_stack = ExitStack()
            xpt0_pool = xpt0_stack.enter_context(
                tc.tile_pool(name="xpt0", bufs=4)
            )
            # interleave w1 quarters with xpt block-0 quarters on the queue;
            # consts ride behind the two first-matmul-critical tiles.
            load_xpt_seg(xpt0_pool, "xpt0", 0, 0, 256)
            nc.sync.dma_start(b1_sb, b1.rearrange("(m p) -> p m", p=128))
            nc.sync.dma_start(w1_sb[1], w1r[:, :, 256:512])
            load_xpt_seg(xpt0_pool, "xpt0", 0, 256, 256)
            nc.sync.dma_start(w1_sb[2], w1r[:, :, 512:768])
            load_xpt_seg(xpt0_pool, "xpt0", 0, 512, 256)
            nc.sync.dma_start(w1_sb[3], w1r[:, :, 768:1024])
            load_xpt_seg(xpt0_pool, "xpt0", 0, 768, 256)
            nc.sync.dma_start(identh_sb, identh)
            nc.sync.dma_start(maskh_sb, maskh)
            # remaining big DMAs, in priority order behind xpt block 0
            wkv_sb = wpool.tile([128, 8, 2 * C], F16, name="wkv_sb")
            nc.sync.dma_start(wkv_sb, wkv.rearrange("(k p) m -> p k m", p=128))
            w2_sb = wpool.tile([128, 8, C], F16, name="w2_sb")
            nc.sync.dma_start(w2_sb, w2.rearrange("(k p) m -> p k m", p=128))
            load_xqt(0)
            load_xqt(1)

            # ctx matmuls are emitted one chunk late so their vector-produced
            # inputs (k, vg) are ready by the time PE reaches them.
            pending = []

            def emit_ctx():
                if not pending:
                    return
                k_sb, vg, nch = pending.pop(0)
                ctp = ctps_pool.tile([128, 1024], F32, name="ctp", tag="ctp")
                for j in range(8):
                    nc.tensor.matmul(
                        ctp[:, j * 128:(j + 1) * 128],
                        vg[:, j * 128:(j + 1) * 128],
                        k_sb[:, j * 128:(j + 1) * 128],
                        start=True,
                        stop=True,
                        skip_group_check=True,
                    )
                nc.vector.tensor_add(ctx_acc, ctx_acc, ctp)

            for blk in range(NBLK):
                if blk + 1 < NBLK:
                    load_xpt_seg(xpt_pool, "xpt", blk + 1, 0, 512)
                    load_xpt_seg(xpt_pool, "xpt", blk + 1, 512, 512)
                if blk + 2 < NBLK:
                    load_xqt(blk + 2)
                segs = xpt_tiles.pop(blk)
                # gate1: hT[m, n] = relu((xp @ w1 + b1).T), w1 stationary;
                # segment-outer so each psum group needs only one xpT segment.
                ht = ht_pool.tile([128, 8, BN], F16, name="ht", tag="ht")
                for (xt, lo, ncols) in segs:
                    for m in range(8):
                        ps = g1ps_pool.tile([128, 512], F32, name="g1ps", tag="g1ps")
                        for kk in range(8):
                            nc.tensor.matmul(
                                ps[:, 0:ncols],
                                w1_sb[m // 2][:, kk, (m % 2) * 128:(m % 2 + 1) * 128],
                                xt[:, kk, :],
                                start=(kk == 0),
                                stop=(kk == 7),
                            )
                        nc.scalar.activation(
                            ht[:, m, lo:lo + ncols],
                            ps[:, 0:ncols],
                            AF.Relu,
                            bias=b1_sb[:, m:m + 1],
                        )
                for ch in range(BCH):
                    nch = blk * BCH + ch
                    seg_i = (ch * 128) // segs[0][2]
                    xpc_t = segs[seg_i][0]
                    xc0 = ch * 128 - segs[seg_i][1]
                    # gate2: g[n, q] = sigmoid(h @ w2 + b2), hT stationary
                    g = g_pool.tile([128, C], F16, name="g", tag="g")
                    for half in range(2):
                        ps = g2ps_pool.tile([128, 512], F32, name="g2ps", tag="g2ps")
                        for kk in range(8):
                            nc.tensor.matmul(
                                ps,
                                ht[:, kk, ch * 128:(ch + 1) * 128],
                                w2_sb[:, kk, half * 512:(half + 1) * 512],
                                start=(kk == 0),
                                stop=(kk == 7 and not with_bias),
                            )
                        if with_bias:
                            nc.tensor.matmul(
                                ps,
                                ones_r,
                                b2_r[:, half * 512:(half + 1) * 512],
                                start=False,
                                stop=True,
                            )
                        nc.scalar.activation(
                            g[:, half * 512:(half + 1) * 512], ps, AF.Sigmoid
                        )
                    # kv projection: kv[n, m], xpT stationary
                    k_sb = k_pool.tile([128, C], F16, name="k_sb", tag="k_sb")
                    vg = vg_pool.tile([128, C], F16, name="vg", tag="vg")
                    for q in range(4):
                        ps = kvps_pool.tile([128, 512], F32, name="kvps", tag="kvps")
                        for kk in range(8):
                            nc.tensor.matmul(
                                ps,
                                xpc_t[:, kk, xc0:xc0 + 128],
                                wkv_sb[:, kk, q * 512:(q + 1) * 512],
                                start=(kk == 0),
                                stop=(kk == 7),
                            )
                        if q < 2:
                            # last chunk: keep DVE free so softmax starts sooner
                            if nch == N // 128 - 1:
                                nc.scalar.copy(k_sb[:, q * 512:(q + 1) * 512], ps)
                            else:
                                nc.vector.tensor_copy(k_sb[:, q * 512:(q + 1) * 512], ps)
                        else:
                            qq = q - 2
                            nc.vector.tensor_mul(
                                vg[:, qq * 512:(qq + 1) * 512],
                                ps,
                                g[:, qq * 512:(qq + 1) * 512],
                            )
                    emit_ctx()
                    pending.append((k_sb, vg, nch))
                if blk == 0:
                    xpt0_stack.close()
            emit_ctx()

        # =========================================================
        # Softmax over d (free dim of ctx^T) + build block-diag S pairs
        # =========================================================
        with ExitStack() as sm:
            smp = sm.enter_context(tc.tile_pool(name="smpool", bufs=1))
            smps = sm.enter_context(tc.tile_pool(name="smps", bufs=2, space="PSUM"))
            # softmax chain pipelined in four 256-col quarters across DVE/ScalarE
            maxs = smp.tile([128, 16], F32, name="maxs")
            cmx = smp.tile([128, 1024], F32, name="cmx")
            et = smp.tile([128, 1024], F32, name="et")
            sums = smp.tile([128, 16], F32, name="sums")
            recs = smp.tile([128, 16], F32, name="recs")
            stb = smp.tile([128, 1024], F16, name="stb")
            for h in range(4):
                sl = slice(h * 256, (h + 1) * 256)
                gsl = slice(h * 4, (h + 1) * 4)
                nc.vector.tensor_reduce(
                    maxs[:, gsl],
                    ctx_acc[:, sl].rearrange("p (g d) -> p g d", g=4),
                    axis=mybir.AxisListType.X,
                    op=mybir.AluOpType.max,
                )
                nc.vector.tensor_sub(
                    cmx[:, sl].rearrange("p (g d) -> p g d", g=4),
                    ctx_acc[:, sl].rearrange("p (g d) -> p g d", g=4),
                    maxs[:, gsl].unsqueeze(-1).broadcast_to([128, 4, 64]),
                )
                nc.scalar.activation(et[:, sl], cmx[:, sl], AF.Exp,
                                     scale=float(SCALE))
                nc.vector.tensor_reduce(
                    sums[:, gsl],
                    et[:, sl].rearrange("p (g d) -> p g d", g=4),
                    axis=mybir.AxisListType.X,
                    op=mybir.AluOpType.add,
                )
                nc.vector.reciprocal(recs[:, gsl], sums[:, gsl])
                nc.vector.tensor_mul(
                    stb[:, sl].rearrange("p (g d) -> p g d", g=4),
                    et[:, sl].rearrange("p (g d) -> p g d", g=4),
                    recs[:, gsl].unsqueeze(-1).broadcast_to([128, 4, 64]),
                )
                # stb[:, j*128:(j+1)*128] = softmaxed ctxT pair [e(2), d(2)];
                # transpose -> [d(2), e(2)], mask off off-diagonal garbage.
                for j in range(2 * h, 2 * h + 2):
                    tp = smps.tile([128, 128], F16, name="smtp", tag="smtp")
                    nc.tensor.transpose(
                        tp, stb[:, j * 128:(j + 1) * 128], identh_sb
                    )
                    nc.vector.tensor_mul(spairs[j], tp, maskh_sb)

        # =========================================================
        # Phase B (transposed out): oT[j*128:(j+1)*128, nblk] =
        #   spair_j^T @ xqT[j-pair rows, nblk].  spair_j stays stationary
        #   across all n (8 LDWEIGHTS total), xqT streams 1024 cols/MM.
        # =========================================================
        with ExitStack() as pb:
            oo_pool = pb.enter_context(tc.tile_pool(name="oo", bufs=3))
            bops_pool = pb.enter_context(
                tc.tile_pool(name="bops", bufs=8, space="PSUM")
            )
            for j in range(8):
                # one oT row-block [128, N] per head pair, two half DMAs out
                oo = oo_pool.tile([128, N], F16, name="oo", tag="oo")
                for blk in range(NBLK):
                    xqt = xqt_tiles[blk]
                    for h in range(2):
                        ops = bops_pool.tile([128, 512], F32, name="ops",
                                             tag="ops")
                        nc.tensor.matmul(
                            ops,
                            spairs[j],
                            xqt[:, j, h * 512:(h + 1) * 512],
                            start=True,
                            stop=True,
                        )
                        lo = blk * BN + h * 512
                        if (blk * 2 + h) % 2 == 0:
                            nc.vector.tensor_copy(oo[:, lo:lo + 512], ops)
                        else:
                            nc.scalar.copy(oo[:, lo:lo + 512], ops)
                    if blk % 2 == 1:
                        nc.sync.dma_start(
                            o[j * 128:(j + 1) * 128,
                              (blk - 1) * BN:(blk + 1) * BN],
                            oo[:, (blk - 1) * BN:(blk + 1) * BN],
                        )

    nc.compile()
    return nc


def _get_program(with_bias=False):
    key = ("nc", bool(with_bias))
    if key not in _CACHE:
        _CACHE[key] = _build_program(with_bias)
    return _CACHE[key]


def make_in_maps(x1, x2, Wkv1, Wkv2, g1_w1, g1_b1, g1_w2, g1_b2,
                 g2_w1, g2_b1, g2_w2, g2_b2):
    """Core (s, b): cores 0-3 = (s=0, b), cores 4-7 = (s=1, b)."""
    f16 = np.float16
    ident = np.eye(128, dtype=f16)
    mask = np.zeros((128, 128), dtype=f16)
    mask[:64, :64] = np.float16(1.0)
    mask[64:, 64:] = np.float16(1.0)
    # transposed fp16 copies of each batch of each stream (shared across cores)
    x1T = [np.asarray(x1[b], np.float32).T.astype(f16) for b in range(x1.shape[0])]
    x2T = [np.asarray(x2[b], np.float32).T.astype(f16) for b in range(x2.shape[0])]
    wkv1h = np.asarray(Wkv1, np.float32).astype(f16)
    wkv2h = np.asarray(Wkv2, np.float32).astype(f16)
    w11h = np.asarray(g1_w1, np.float32).astype(f16)
    w12h = np.asarray(g1_w2, np.float32).astype(f16)
    w21h = np.asarray(g2_w1, np.float32).astype(f16)
    w22h = np.asarray(g2_w2, np.float32).astype(f16)
    b11 = np.asarray(g1_b1, np.float32)
    b21 = np.asarray(g2_b1, np.float32)
    b12h = np.asarray(g1_b2, np.float32).astype(f16)
    b22h = np.asarray(g2_b2, np.float32).astype(f16)
    in_maps = []
    for core in range(8):
        s, b = core // 4, core % 4
        if s == 0:
            m = dict(xpT=x1T[b], xqT=x2T[b], wkv=wkv1h,
                     w1=w11h, b1=b11, w2=w12h, b2=b12h)
        else:
            m = dict(xpT=x2T[b], xqT=x1T[b], wkv=wkv2h,
                     w1=w21h, b1=b21, w2=w22h, b2=b22h)
        m["identh"] = ident
        m["maskh"] = mask
        in_maps.append(m)
    return in_maps


def kernel(x1, x2, Wkv1, Wkv2, g1_w1, g1_b1, g1_w2, g1_b2,
           g2_w1, g2_b1, g2_w2, g2_b2, _runner=None):
    """Full-input entry point.  Returns (o1, o2), each [4, 4096, 1024] f32."""
    from concourse.bass_utils import run_bass_kernel_spmd

    args = [np.asarray(a, dtype=np.float32) for a in
            (x1, x2, Wkv1, Wkv2, g1_w1, g1_b1, g1_w2, g1_b2,
             g2_w1, g2_b1, g2_w2, g2_b2)]
    with_bias = bool(np.any(args[7]) or np.any(args[11]))  # g1_b2, g2_b2
    nc = _get_program(with_bias)
    in_maps = make_in_maps(*args)
    if _runner is None:
        res = run_bass_kernel_spmd(nc, in_maps, core_ids=list(range(8)))
        results = res.results
    else:
        results = _runner(nc, in_maps)

    B = x1.shape[0]
    o1 = np.empty((B, N, C), dtype=np.float32)
    o2 = np.empty((B, N, C), dtype=np.float32)
    for core in range(8):
        s, b = core // 4, core % 4
        out = np.asarray(results[core]["o"]).T.astype(np.float32)  # [C,N] -> [N,C]
        if s == 0:
            o2[b] = out   # core projected x1 -> ctx1 -> o2 = q2 @ ctx1
        else:
            o1[b] = out
    return (o1, o2)


# revision 44
# speedup vs baseline: 1.0103x; 1.0103x over previous
"""Trainium2 Bass kernel for nn_CrossAttention (dense_transformer).

Reference computation (per batch b, per stream s in {1,2}):
    q_s   = heads(x_s)                      # [H, N, D] slices of x_s
    kv_s  = x_s @ Wkv_s -> k_s, v_s         # [N, C] each
    gate_s= sigmoid(relu(x_s @ w1 + b1) @ w2 + b2)
    ctx_s = softmax_d( scale * k_s^T @ (v_s * gate_s) )   # [H, D, D], softmax over d
    o_1   = q_1 @ ctx_2 ; o_2 = q_2 @ ctx_1  (cross)

Sharding: 8 cores = (stream s, batch b) pairs.  Core (s, b) projects
x_s[b] (kv + gate + ctx_s[b]) and then computes the OTHER stream's
output o_{1-s}[b] = q_{1-s}[b] @ softmax(ctx_s[b]).  No cross-core
communication; host concatenates outputs.

v2: host pre-transposes and fp16-casts x (so no on-chip transposes),
fp16 matmul operands everywhere (fp32 PSUM accumulate), fused
block-pipelined gate1 -> gate2/kv/vg -> ctx (per-chunk one-shot PSUM
partials DVE-accumulated in SBUF), pipelined softmax, then phase B
with spair-stationary wide matmuls producing o transposed (host
un-transposes).  xqT is fully prefetched during phase A so phase B's
DMA is writes-only.
"""

import numpy as np
from contextlib import ExitStack

N = 4096
C = 1024
H = 16
D = 64
SCALE = D ** (-0.5)
NBLK = 4            # n-blocks of 1024 rows
BN = N // NBLK      # 1024 rows per block
BCH = BN // 128     # 8 chunks of 128 rows per block

_CACHE = {}


def _build_program(with_bias):
    """Build the SPMD Bass program (same for all 8 cores)."""
    import concourse.bass as bass
    import concourse.bacc as bacc
    import concourse.tile as tile
    import concourse.mybir as mybir

    F32 = mybir.dt.float32
    F16 = mybir.dt.float16
    AF = mybir.ActivationFunctionType

    nc = bacc.Bacc("TRN2", target_bir_lowering=False, debug=False, num_devices=8)

    # inputs are host-pre-tiled into the exact SBUF layouts so every DMA is a
    # single contiguous run per partition (fewest descriptors):
    #   xpq: block-0 quarters [4, 128p, 8k, 256n]
    #   xph: blocks 1-3 halves [6, 128p, 8k, 512n]
    #   xqb: xqT blocks       [4, 128p, 8k, 1024n]
    #   w1q: w1 m-quarters    [4, 128p, 8k, 256m]
    #   wkvp/w2p:             [128p, 8k, 2048/1024 m]
    xpq = [nc.dram_tensor(f"xpq{i}", [128, 8, 256], F16,
                          kind="ExternalInput").ap() for i in range(4)]
    xph = [nc.dram_tensor(f"xph{i}", [128, 8, 512], F16,
                          kind="ExternalInput").ap() for i in range(6)]
    xqb = [nc.dram_tensor(f"xqb{i}", [128, 8, BN], F16,
                          kind="ExternalInput").ap() for i in range(4)]
    wkvp = nc.dram_tensor("wkvp", [128, 8, 2 * C], F16, kind="ExternalInput").ap()
    w1q = [nc.dram_tensor(f"w1q{i}", [128, 8, 256], F16,
                          kind="ExternalInput").ap() for i in range(4)]
    w2p = nc.dram_tensor("w2p", [128, 8, C], F16, kind="ExternalInput").ap()
    b1 = nc.dram_tensor("b1", [C], F32, kind="ExternalInput").ap()
    b2 = nc.dram_tensor("b2", [C], F16, kind="ExternalInput").ap()
    identh = nc.dram_tensor("identh", [128, 128], F16, kind="ExternalInput").ap()
    maskh = nc.dram_tensor("maskh", [128, 128], F16, kind="ExternalInput").ap()
    # output is produced TRANSPOSED ([channel, n]); host un-transposes.
    o = nc.dram_tensor("o", [C, N], F16, kind="ExternalOutput").ap()

    with tile.TileContext(nc) as tc, ExitStack() as ctx:
        # ---------- persistent pools ----------
        cpool = ctx.enter_context(tc.tile_pool(name="consts", bufs=1))
        b1_sb = cpool.tile([128, 8], F32, name="b1_sb")  # b1_sb[p, m] = b1[m*128+p]
        identh_sb = cpool.tile([128, 128], F16, name="identh_sb")
        maskh_sb = cpool.tile([128, 128], F16, name="maskh_sb")
        if with_bias:
            ones_r = cpool.tile([1, 128], F16, name="ones_r")
            nc.vector.memset(ones_r, 1.0)
            b2_r = cpool.tile([1, C], F16, name="b2_r")
            nc.sync.dma_start(b2_r, b2.rearrange("(one f) -> one f", one=1))

        spool = ctx.enter_context(tc.tile_pool(name="spairs", bufs=1))
        spairs = [spool.tile([128, 128], F16, name=f"spair{j}") for j in range(8)]
        # preload the ScalarE Exp table now so the softmax-critical-path
        # activation doesn't pay the ~1.5us ACT_TABLE_LOAD later.
        expwarm = cpool.tile([1, 1], F32, name="expwarm")
        expwarm2 = cpool.tile([1, 1], F32, name="expwarm2")
        nc.vector.memset(expwarm, 0.0)
        nc.scalar.activation(expwarm2, expwarm, AF.Exp)

        # weights: w1 first (gate1 is the critical path at startup), split in
        # m-quarters so gate1 m=0 can start before the whole matrix lands.
        wpool = ctx.enter_context(tc.tile_pool(name="weights", bufs=1))
        w1_sb = [wpool.tile([128, 8, C // 4], F16, name=f"w1_sb{qq}")
                 for qq in range(4)]
        nc.sync.dma_start(w1_sb[0], w1q[0])

        # ctx^T accumulator in SBUF, [e(2-head pair), d(2-head pair)] per pair j
        # at cols j*128; per-chunk partials land in a PSUM tile and are
        # DVE-accumulated here (one-shot matmul groups, like the baseline).
        acc_pool = ctx.enter_context(tc.tile_pool(name="ctxacc", bufs=1))
        ctx_acc = acc_pool.tile([128, 1024], F32, name="ctx_acc")
        nc.vector.memset(ctx_acc, 0.0)

        # xqT prefetch pool (phase B input), fully resident by end of phase A
        # so phase B's DMA is writes-only (reads+writes together oversubscribe
        # the 358 GB/s DMA during the short phase B window).
        xqt_pool = ctx.enter_context(tc.tile_pool(name="xqt", bufs=4))
        xqt_tiles = {}

        def load_xqt(blk):
            t = xqt_pool.tile([128, 8, BN], F16, name="xqt", tag="xqt")
            nc.sync.dma_start(t, xqb[blk])
            xqt_tiles[blk] = t

        # =========================================================
        # Phase A: gate MLP + kv projection + ctx accumulation,
        # fused per n-block of 1024 rows.
        # =========================================================
        with ExitStack() as pa:
            # xpT arrives in segments (block 0: four 256-col quarters so the
            # first gate1 group starts ~3us in; later blocks: 512-col halves).
            xpt_pool = pa.enter_context(tc.tile_pool(name="xpt", bufs=2))
            xpt_tiles = {}

            def load_xpt_seg(pool, tag, blk, lo, ncols):
                t = pool.tile([128, 8, ncols], F16, name=tag, tag=tag)
                if ncols == 256:
                    src = xpq[lo // 256]
                else:
                    src = xph[(blk - 1) * 2 + lo // 512]
                nc.sync.dma_start(t, src)
                xpt_tiles.setdefault(blk, []).append((t, lo, ncols))

            ht_pool = pa.enter_context(tc.tile_pool(name="ht", bufs=1))
            g_pool = pa.enter_context(tc.tile_pool(name="g", bufs=2))
            k_pool = pa.enter_context(tc.tile_pool(name="k", bufs=2))
            vg_pool = pa.enter_context(tc.tile_pool(name="vg", bufs=2))
            g1ps_pool = pa.enter_context(
                tc.tile_pool(name="g1ps", bufs=2, space="PSUM")
            )
            g2ps_pool = pa.enter_context(
                tc.tile_pool(name="g2ps", bufs=2, space="PSUM")
            )
            kvps_pool = pa.enter_context(
                tc.tile_pool(name="kvps", bufs=2, space="PSUM")
            )
            ctps_pool = pa.enter_context(
                tc.tile_pool(name="ctps", bufs=1, space="PSUM")
            )
            # block-0 quarter tiles live only until block 0 is consumed
            xpt0_stack = ExitStack()
            xpt0_pool = xpt0_stack.enter_context(
                tc.tile_pool(name="xpt0", bufs=4)
            )
            # interleave w1 quarters with xpt block-0 quarters on the queue;
            # consts ride behind the two first-matmul-critical tiles.
            load_xpt_seg(xpt0_pool, "xpt0", 0, 0, 256)
            nc.sync.dma_start(b1_sb, b1.rearrange("(m p) -> p m", p=128))
            nc.sync.dma_start(w1_sb[1], w1q[1])
            load_xpt_seg(xpt0_pool, "xpt0", 0, 256, 256)
            nc.sync.dma_start(w1_sb[2], w1q[2])
            load_xpt_seg(xpt0_pool, "xpt0", 0, 512, 256)
            nc.sync.dma_start(w1_sb[3], w1q[3])
            load_xpt_seg(xpt0_pool, "xpt0", 0, 768, 256)
            nc.sync.dma_start(identh_sb, identh)
            nc.sync.dma_start(maskh_sb, maskh)
            # remaining big DMAs, in priority order behind xpt block 0
            wkv_sb = wpool.tile([128, 8, 2 * C], F16, name="wkv_sb")
            nc.sync.dma_start(wkv_sb, wkvp)
            w2_sb = wpool.tile([128, 8, C], F16, name="w2_sb")
            nc.sync.dma_start(w2_sb, w2p)
            load_xqt(0)
            load_xqt(1)

            # ctx matmuls are emitted one chunk late so their vector-produced
            # inputs (k, vg) are ready by the time PE reaches them.
            pending = []

            def emit_ctx():
                if not pending:
                    return
                k_sb, vg, nch = pending.pop(0)
                ctp = ctps_pool.tile([128, 1024], F32, name="ctp", tag="ctp")
                for j in range(8):
                    nc.tensor.matmul(
                        ctp[:, j * 128:(j + 1) * 128],
                        vg[:, j * 128:(j + 1) * 128],
                        k_sb[:, j * 128:(j + 1) * 128],
                        start=True,
                        stop=True,
                        skip_group_check=True,
                    )
                nc.vector.tensor_add(ctx_acc, ctx_acc, ctp)

            for blk in range(NBLK):
                if blk + 1 < NBLK:
                    load_xpt_seg(xpt_pool, "xpt", blk + 1, 0, 512)
                    load_xpt_seg(xpt_pool, "xpt", blk + 1, 512, 512)
                if blk + 2 < NBLK:
                    load_xqt(blk + 2)
                segs = xpt_tiles.pop(blk)
                # gate1: hT[m, n] = relu((xp @ w1 + b1).T), w1 stationary;
                # segment-outer so each psum group needs only one xpT segment.
                ht = ht_pool.tile([128, 8, BN], F16, name="ht", tag="ht")
                for (xt, lo, ncols) in segs:
                    for m in range(8):
                        ps = g1ps_pool.tile([128, 512], F32, name="g1ps", tag="g1ps")
                        for kk in range(8):
                            nc.tensor.matmul(
                                ps[:, 0:ncols],
                                w1_sb[m // 2][:, kk, (m % 2) * 128:(m % 2 + 1) * 128],
                                xt[:, kk, :],
                                start=(kk == 0),
                                stop=(kk == 7),
                            )
                        nc.scalar.activation(
                            ht[:, m, lo:lo + ncols],
                            ps[:, 0:ncols],
                            AF.Relu,
                            bias=b1_sb[:, m:m + 1],
                        )
                for ch in range(BCH):
                    nch = blk * BCH + ch
                    seg_i = (ch * 128) // segs[0][2]
                    xpc_t = segs[seg_i][0]
                    xc0 = ch * 128 - segs[seg_i][1]
                    # gate2: g[n, q] = sigmoid(h @ w2 + b2), hT stationary
                    g = g_pool.tile([128, C], F16, name="g", tag="g")
                    for half in range(2):
                        ps = g2ps_pool.tile([128, 512], F32, name="g2ps", tag="g2ps")
                        for kk in range(8):
                            nc.tensor.matmul(
                                ps,
                                ht[:, kk, ch * 128:(ch + 1) * 128],
                                w2_sb[:, kk, half * 512:(half + 1) * 512],
                                start=(kk == 0),
                                stop=(kk == 7 and not with_bias),
                            )
                        if with_bias:
                            nc.tensor.matmul(
                                ps,
                                ones_r,
                                b2_r[:, half * 512:(half + 1) * 512],
                                start=False,
                                stop=True,
                            )
                        nc.scalar.activation(
                            g[:, half * 512:(half + 1) * 512], ps, AF.Sigmoid
                        )
                    # kv projection: kv[n, m], xpT stationary
                    k_sb = k_pool.tile([128, C], F16, name="k_sb", tag="k_sb")
                    vg = vg_pool.tile([128, C], F16, name="vg", tag="vg")
                    for q in range(4):
                        ps = kvps_pool.tile([128, 512], F32, name="kvps", tag="kvps")
                        for kk in range(8):
                            nc.tensor.matmul(
                                ps,
                                xpc_t[:, kk, xc0:xc0 + 128],
                                wkv_sb[:, kk, q * 512:(q + 1) * 512],
                                start=(kk == 0),
                                stop=(kk == 7),
                            )
                        if q < 2:
                            # last chunk: keep DVE free so softmax starts sooner
                            if nch == N // 128 - 1:
                                nc.scalar.copy(k_sb[:, q * 512:(q + 1) * 512], ps)
                            else:
                                nc.vector.tensor_copy(k_sb[:, q * 512:(q + 1) * 512], ps)
                        else:
                            qq = q - 2
                            nc.vector.tensor_mul(
                                vg[:, qq * 512:(qq + 1) * 512],
                                ps,
                                g[:, qq * 512:(qq + 1) * 512],
                            )
                    emit_ctx()
                    pending.append((k_sb, vg, nch))
                if blk == 0:
                    xpt0_stack.close()
            emit_ctx()

        # =========================================================
        # Softmax over d (free dim of ctx^T) + build block-diag S pairs
        # =========================================================
        with ExitStack() as sm:
            smp = sm.enter_context(tc.tile_pool(name="smpool", bufs=1))
            smps = sm.enter_context(tc.tile_pool(name="smps", bufs=2, space="PSUM"))
            # softmax chain pipelined in four 256-col quarters across DVE/ScalarE
            maxs = smp.tile([128, 16], F32, name="maxs")
            cmx = smp.tile([128, 1024], F32, name="cmx")
            et = smp.tile([128, 1024], F32, name="et")
            sums = smp.tile([128, 16], F32, name="sums")
            recs = smp.tile([128, 16], F32, name="recs")
            stb = smp.tile([128, 1024], F16, name="stb")
            for h in range(4):
                sl = slice(h * 256, (h + 1) * 256)
                gsl = slice(h * 4, (h + 1) * 4)
                nc.vector.tensor_reduce(
                    maxs[:, gsl],
                    ctx_acc[:, sl].rearrange("p (g d) -> p g d", g=4),
                    axis=mybir.AxisListType.X,
                    op=mybir.AluOpType.max,
                )
                nc.vector.tensor_sub(
                    cmx[:, sl].rearrange("p (g d) -> p g d", g=4),
                    ctx_acc[:, sl].rearrange("p (g d) -> p g d", g=4),
                    maxs[:, gsl].unsqueeze(-1).broadcast_to([128, 4, 64]),
                )
                nc.scalar.activation(et[:, sl], cmx[:, sl], AF.Exp,
                                     scale=float(SCALE))
                nc.vector.tensor_reduce(
                    sums[:, gsl],
                    et[:, sl].rearrange("p (g d) -> p g d", g=4),
                    axis=mybir.AxisListType.X,
                    op=mybir.AluOpType.add,
                )
                nc.vector.reciprocal(recs[:, gsl], sums[:, gsl])
                nc.vector.tensor_mul(
                    stb[:, sl].rearrange("p (g d) -> p g d", g=4),
                    et[:, sl].rearrange("p (g d) -> p g d", g=4),
                    recs[:, gsl].unsqueeze(-1).broadcast_to([128, 4, 64]),
                )
                # stb[:, j*128:(j+1)*128] = softmaxed ctxT pair [e(2), d(2)];
                # transpose -> [d(2), e(2)], mask off off-diagonal garbage.
                for j in range(2 * h, 2 * h + 2):
                    tp = smps.tile([128, 128], F16, name="smtp", tag="smtp")
                    nc.tensor.transpose(
                        tp, stb[:, j * 128:(j + 1) * 128], identh_sb
                    )
                    nc.vector.tensor_mul(spairs[j], tp, maskh_sb)

        # =========================================================
        # Phase B (transposed out): oT[j*128:(j+1)*128, nblk] =
        #   spair_j^T @ xqT[j-pair rows, nblk].  spair_j stays stationary
        #   across all n (8 LDWEIGHTS total), xqT streams 1024 cols/MM.
        # =========================================================
        with ExitStack() as pb:
            oo_pool = pb.enter_context(tc.tile_pool(name="oo", bufs=3))
            bops_pool = pb.enter_context(
                tc.tile_pool(name="bops", bufs=8, space="PSUM")
            )
            for j in range(8):
                # one oT row-block [128, N] per head pair, two half DMAs out
                oo = oo_pool.tile([128, N], F16, name="oo", tag="oo")
                for blk in range(NBLK):
                    xqt = xqt_tiles[blk]
                    for h in range(2):
                        ops = bops_pool.tile([128, 512], F32, name="ops",
                                             tag="ops")
                        nc.tensor.matmul(
                            ops,
                            spairs[j],
                            xqt[:, j, h * 512:(h + 1) * 512],
                            start=True,
                            stop=True,
                        )
                        lo = blk * BN + h * 512
                        if (blk * 2 + h) % 2 == 0:
                            nc.vector.tensor_copy(oo[:, lo:lo + 512], ops)
                        else:
                            nc.scalar.copy(oo[:, lo:lo + 512], ops)
                    if blk % 2 == 1:
                        nc.sync.dma_start(
                            o[j * 128:(j + 1) * 128,
                              (blk - 1) * BN:(blk + 1) * BN],
                            oo[:, (blk - 1) * BN:(blk + 1) * BN],
                        )

    nc.compile()
    return nc


def _get_program(with_bias=False):
    key = ("nc", bool(with_bias))
    if key not in _CACHE:
        _CACHE[key] = _build_program(with_bias)
    return _CACHE[key]


def make_in_maps(x1, x2, Wkv1, Wkv2, g1_w1, g1_b1, g1_w2, g1_b2,
                 g2_w1, g2_b1, g2_w2, g2_b2):
    """Core (s, b): cores 0-3 = (s=0, b), cores 4-7 = (s=1, b)."""
    f16 = np.float16
    ident = np.eye(128, dtype=f16)
    mask = np.zeros((128, 128), dtype=f16)
    mask[:64, :64] = np.float16(1.0)
    mask[64:, 64:] = np.float16(1.0)
    asc = np.ascontiguousarray

    def ctile(a2d, nseg, ncols):
        # [C, W] -> SBUF layout segments [nseg, 128p, 8k, ncols]
        t = a2d.reshape(8, 128, a2d.shape[1]).transpose(1, 0, 2)  # [p, k, W]
        return asc(t.reshape(128, 8, nseg, ncols).transpose(2, 0, 1, 3))

    def xforms(x):
        # per batch: transposed fp16, pre-tiled as blk-0 quarters, halves 1-3,
        # and full xq blocks
        xT = np.asarray(x, np.float32).T.astype(f16)  # [C, N]
        base = xT.reshape(8, 128, N).transpose(1, 0, 2)  # [p, k, n]
        xq_b = asc(base.reshape(128, 8, 4, BN).transpose(2, 0, 1, 3))
        xp_q = asc(base[:, :, 0:BN].reshape(128, 8, 4, 256).transpose(2, 0, 1, 3))
        xp_h = asc(base[:, :, BN:].reshape(128, 8, 6, 512).transpose(2, 0, 1, 3))
        return xp_q, xp_h, xq_b

    x1f = [xforms(x1[b]) for b in range(x1.shape[0])]
    x2f = [xforms(x2[b]) for b in range(x2.shape[0])]
    wkv1p = ctile(np.asarray(Wkv1, np.float32).astype(f16), 1, 2048)[0]
    wkv2p = ctile(np.asarray(Wkv2, np.float32).astype(f16), 1, 2048)[0]
    w11q = ctile(np.asarray(g1_w1, np.float32).astype(f16), 4, 256)
    w21q = ctile(np.asarray(g2_w1, np.float32).astype(f16), 4, 256)
    w12p = ctile(np.asarray(g1_w2, np.float32).astype(f16), 1, 1024)[0]
    w22p = ctile(np.asarray(g2_w2, np.float32).astype(f16), 1, 1024)[0]
    b11 = np.asarray(g1_b1, np.float32)
    b21 = np.asarray(g2_b1, np.float32)
    b12h = np.asarray(g1_b2, np.float32).astype(f16)
    b22h = np.asarray(g2_b2, np.float32).astype(f16)
    in_maps = []
    for core in range(8):
        s, b = core // 4, core % 4
        if s == 0:
            xpf, xqf = x1f[b], x2f[b]
            m = dict(wkvp=wkv1p, b1=b11, w2p=w12p, b2=b12h)
            w1s = w11q
        else:
            xpf, xqf = x2f[b], x1f[b]
            m = dict(wkvp=wkv2p, b1=b21, w2p=w22p, b2=b22h)
            w1s = w21q
        for i in range(4):
            m[f"xpq{i}"] = np.ascontiguousarray(xpf[0][i])
            m[f"xqb{i}"] = np.ascontiguousarray(xqf[2][i])
            m[f"w1q{i}"] = np.ascontiguousarray(w1s[i])
        for i in range(6):
            m[f"xph{i}"] = np.ascontiguousarray(xpf[1][i])
        m["identh"] = ident
        m["maskh"] = mask
        in_maps.append(m)
    return in_maps


def kernel(x1, x2, Wkv1, Wkv2, g1_w1, g1_b1, g1_w2, g1_b2,
           g2_w1, g2_b1, g2_w2, g2_b2, _runner=None):
    """Full-input entry point.  Returns (o1, o2), each [4, 4096, 1024] f32."""
    from concourse.bass_utils import run_bass_kernel_spmd

    args = [np.asarray(a, dtype=np.float32) for a in
            (x1, x2, Wkv1, Wkv2, g1_w1, g1_b1, g1_w2, g1_b2,
             g2_w1, g2_b1, g2_w2, g2_b2)]
    with_bias = bool(np.any(args[7]) or np.any(args[11]))  # g1_b2, g2_b2
    nc = _get_program(with_bias)
    in_maps = make_in_maps(*args)
    if _runner is None:
        res = run_bass_kernel_spmd(nc, in_maps, core_ids=list(range(8)))
        results = res.results
    else:
        results = _runner(nc, in_maps)

    B = x1.shape[0]
    o1 = np.empty((B, N, C), dtype=np.float32)
    o2 = np.empty((B, N, C), dtype=np.float32)
    for core in range(8):
        s, b = core // 4, core % 4
        out = np.asarray(results[core]["o"]).T.astype(np.float32)  # [C,N] -> [N,C]
        if s == 0:
            o2[b] = out   # core projected x1 -> ctx1 -> o2 = q2 @ ctx1
        else:
            o1[b] = out
    return (o1, o2)


# revision 49
# speedup vs baseline: 1.0134x; 1.0031x over previous
"""Trainium2 Bass kernel for nn_CrossAttention (dense_transformer).

Reference computation (per batch b, per stream s in {1,2}):
    q_s   = heads(x_s)                      # [H, N, D] slices of x_s
    kv_s  = x_s @ Wkv_s -> k_s, v_s         # [N, C] each
    gate_s= sigmoid(relu(x_s @ w1 + b1) @ w2 + b2)
    ctx_s = softmax_d( scale * k_s^T @ (v_s * gate_s) )   # [H, D, D], softmax over d
    o_1   = q_1 @ ctx_2 ; o_2 = q_2 @ ctx_1  (cross)

Sharding: 8 cores = (stream s, batch b) pairs.  Core (s, b) projects
x_s[b] (kv + gate + ctx_s[b]) and then computes the OTHER stream's
output o_{1-s}[b] = q_{1-s}[b] @ softmax(ctx_s[b]).  No cross-core
communication; host concatenates outputs.

v2: host pre-transposes and fp16-casts x (so no on-chip transposes),
fp16 matmul operands everywhere (fp32 PSUM accumulate), fused
block-pipelined gate1 -> gate2/kv/vg -> ctx (per-chunk one-shot PSUM
partials DVE-accumulated in SBUF), pipelined softmax, then phase B
with spair-stationary wide matmuls producing o transposed (host
un-transposes).  xqT is fully prefetched during phase A so phase B's
DMA is writes-only.
"""

import numpy as np
from contextlib import ExitStack

N = 4096
C = 1024
H = 16
D = 64
SCALE = D ** (-0.5)
NBLK = 4            # n-blocks of 1024 rows
BN = N // NBLK      # 1024 rows per block
BCH = BN // 128     # 8 chunks of 128 rows per block

_CACHE = {}


def _build_program(with_bias):
    """Build the SPMD Bass program (same for all 8 cores)."""
    import concourse.bass as bass
    import concourse.bacc as bacc
    import concourse.tile as tile
    import concourse.mybir as mybir

    F32 = mybir.dt.float32
    F16 = mybir.dt.float16
    AF = mybir.ActivationFunctionType

    nc = bacc.Bacc("TRN2", target_bir_lowering=False, debug=False, num_devices=8)

    # inputs are host-pre-tiled into the exact SBUF layouts so every DMA is a
    # single contiguous run per partition (fewest descriptors):
    #   xpq: block-0 quarters [4, 128p, 8k, 256n]
    #   xph: blocks 1-3 halves [6, 128p, 8k, 512n]
    #   xqb: xqT blocks       [4, 128p, 8k, 1024n]
    #   w1q: w1 m-quarters    [4, 128p, 8k, 256m]
    #   wkvp/w2p:             [128p, 8k, 2048/1024 m]
    xpq = [nc.dram_tensor(f"xpq{i}", [128, 8, 256], F16,
                          kind="ExternalInput").ap() for i in range(4)]
    xph = [nc.dram_tensor(f"xph{i}", [128, 8, 512], F16,
                          kind="ExternalInput").ap() for i in range(6)]
    xqb = [nc.dram_tensor(f"xqb{i}", [128, 8, BN], F16,
                          kind="ExternalInput").ap() for i in range(4)]
    wkvp = nc.dram_tensor("wkvp", [128, 8, 2 * C], F16, kind="ExternalInput").ap()
    w1q = [nc.dram_tensor(f"w1q{i}", [128, 8, 256], F16,
                          kind="ExternalInput").ap() for i in range(4)]
    w2p = nc.dram_tensor("w2p", [128, 8, C], F16, kind="ExternalInput").ap()
    b1 = nc.dram_tensor("b1", [C], F32, kind="ExternalInput").ap()
    b2 = nc.dram_tensor("b2", [C], F16, kind="ExternalInput").ap()
    identh = nc.dram_tensor("identh", [128, 128], F16, kind="ExternalInput").ap()
    maskh = nc.dram_tensor("maskh", [128, 128], F16, kind="ExternalInput").ap()
    # output is produced TRANSPOSED ([channel, n]); host un-transposes.
    o = nc.dram_tensor("o", [C, N], F16, kind="ExternalOutput").ap()

    with tile.TileContext(nc) as tc, ExitStack() as ctx:
        # ---------- persistent pools ----------
        cpool = ctx.enter_context(tc.tile_pool(name="consts", bufs=1))
        b1_sb = cpool.tile([128, 8], F32, name="b1_sb")  # b1_sb[p, m] = b1[m*128+p]
        identh_sb = cpool.tile([128, 128], F16, name="identh_sb")
        maskh_sb = cpool.tile([128, 128], F16, name="maskh_sb")
        if with_bias:
            ones_r = cpool.tile([1, 128], F16, name="ones_r")
            nc.vector.memset(ones_r, 1.0)
            b2_r = cpool.tile([1, C], F16, name="b2_r")
            nc.sync.dma_start(b2_r, b2.rearrange("(one f) -> one f", one=1))

        spool = ctx.enter_context(tc.tile_pool(name="spairs", bufs=1))
        spairs = [spool.tile([128, 128], F16, name=f"spair{j}") for j in range(8)]
        # preload the ScalarE Exp table now so the softmax-critical-path
        # activation doesn't pay the ~1.5us ACT_TABLE_LOAD later.
        expwarm = cpool.tile([1, 1], F32, name="expwarm")
        expwarm2 = cpool.tile([1, 1], F32, name="expwarm2")
        nc.vector.memset(expwarm, 0.0)
        nc.scalar.activation(expwarm2, expwarm, AF.Exp)

        # weights: w1 first (gate1 is the critical path at startup), split in
        # m-quarters so gate1 m=0 can start before the whole matrix lands.
        wpool = ctx.enter_context(tc.tile_pool(name="weights", bufs=1))
        w1_sb = [wpool.tile([128, 8, C // 4], F16, name=f"w1_sb{qq}")
                 for qq in range(4)]
        nc.sync.dma_start(w1_sb[0], w1q[0])

        # ctx^T accumulator in SBUF, [e(2-head pair), d(2-head pair)] per pair j
        # at cols j*128; per-chunk partials land in a PSUM tile and are
        # DVE-accumulated here (one-shot matmul groups, like the baseline).
        acc_pool = ctx.enter_context(tc.tile_pool(name="ctxacc", bufs=1))
        ctx_acc = acc_pool.tile([128, 1024], F32, name="ctx_acc")
        nc.vector.memset(ctx_acc, 0.0)

        # xqT prefetch pool (phase B input), fully resident by end of phase A
        # so phase B's DMA is writes-only (reads+writes together oversubscribe
        # the 358 GB/s DMA during the short phase B window).
        xqt_pool = ctx.enter_context(tc.tile_pool(name="xqt", bufs=4))
        xqt_tiles = {}

        def load_xqt(blk):
            t = xqt_pool.tile([128, 8, BN], F16, name="xqt", tag="xqt")
            nc.sync.dma_start(t, xqb[blk])
            xqt_tiles[blk] = t

        # =========================================================
        # Phase A: gate MLP + kv projection + ctx accumulation,
        # fused per n-block of 1024 rows.
        # =========================================================
        with ExitStack() as pa:
            # xpT arrives in segments (block 0: four 256-col quarters so the
            # first gate1 group starts ~3us in; later blocks: 512-col halves).
            xpt_pool = pa.enter_context(tc.tile_pool(name="xpt", bufs=2))
            xpt_tiles = {}

            def load_xpt_seg(pool, tag, blk, lo, ncols):
                t = pool.tile([128, 8, ncols], F16, name=tag, tag=tag)
                if ncols == 256:
                    src = xpq[lo // 256]
                else:
                    src = xph[(blk - 1) * 2 + lo // 512]
                nc.sync.dma_start(t, src)
                xpt_tiles.setdefault(blk, []).append((t, lo, ncols))

            ht_pool = pa.enter_context(tc.tile_pool(name="ht", bufs=1))
            g_pool = pa.enter_context(tc.tile_pool(name="g", bufs=2))
            k_pool = pa.enter_context(tc.tile_pool(name="k", bufs=2))
            vg_pool = pa.enter_context(tc.tile_pool(name="vg", bufs=2))
            g1ps_pool = pa.enter_context(
                tc.tile_pool(name="g1ps", bufs=2, space="PSUM")
            )
            g2ps_pool = pa.enter_context(
                tc.tile_pool(name="g2ps", bufs=2, space="PSUM")
            )
            kvps_pool = pa.enter_context(
                tc.tile_pool(name="kvps", bufs=2, space="PSUM")
            )
            ctps_pool = pa.enter_context(
                tc.tile_pool(name="ctps", bufs=1, space="PSUM")
            )
            # block-0 quarter tiles live only until block 0 is consumed
            xpt0_stack = ExitStack()
            xpt0_pool = xpt0_stack.enter_context(
                tc.tile_pool(name="xpt0", bufs=4)
            )
            # interleave w1 quarters with xpt block-0 quarters on the queue;
            # consts ride behind the two first-matmul-critical tiles.
            load_xpt_seg(xpt0_pool, "xpt0", 0, 0, 256)
            nc.sync.dma_start(b1_sb, b1.rearrange("(m p) -> p m", p=128))
            nc.sync.dma_start(w1_sb[1], w1q[1])
            load_xpt_seg(xpt0_pool, "xpt0", 0, 256, 256)
            nc.sync.dma_start(w1_sb[2], w1q[2])
            load_xpt_seg(xpt0_pool, "xpt0", 0, 512, 256)
            nc.sync.dma_start(w1_sb[3], w1q[3])
            load_xpt_seg(xpt0_pool, "xpt0", 0, 768, 256)
            nc.sync.dma_start(identh_sb, identh)
            nc.sync.dma_start(maskh_sb, maskh)
            # remaining big DMAs, in priority order behind xpt block 0
            wkv_sb = wpool.tile([128, 8, 2 * C], F16, name="wkv_sb")
            nc.sync.dma_start(wkv_sb, wkvp)
            w2_sb = wpool.tile([128, 8, C], F16, name="w2_sb")
            nc.sync.dma_start(w2_sb, w2p)
            load_xqt(0)
            load_xqt(1)

            # ctx matmuls are emitted one chunk late so their vector-produced
            # inputs (k, vg) are ready by the time PE reaches them.
            pending = []

            def emit_ctx():
                if not pending:
                    return
                k_sb, vg, nch = pending.pop(0)
                ctp = ctps_pool.tile([128, 1024], F32, name="ctp", tag="ctp")
                for j in range(8):
                    nc.tensor.matmul(
                        ctp[:, j * 128:(j + 1) * 128],
                        vg[:, j * 128:(j + 1) * 128],
                        k_sb[:, j * 128:(j + 1) * 128],
                        start=True,
                        stop=True,
                        skip_group_check=True,
                    )
                nc.vector.tensor_add(ctx_acc, ctx_acc, ctp)

            for blk in range(NBLK):
                if blk + 1 < NBLK:
                    load_xpt_seg(xpt_pool, "xpt", blk + 1, 0, 512)
                    load_xpt_seg(xpt_pool, "xpt", blk + 1, 512, 512)
                if blk + 2 < NBLK:
                    load_xqt(blk + 2)
                segs = xpt_tiles.pop(blk)
                # gate1: hT[m, n] = relu((xp @ w1 + b1).T), w1 stationary;
                # segment-outer so each psum group needs only one xpT segment.
                ht = ht_pool.tile([128, 8, BN], F16, name="ht", tag="ht")
                for (xt, lo, ncols) in segs:
                    for m in range(8):
                        ps = g1ps_pool.tile([128, 512], F32, name="g1ps", tag="g1ps")
                        for kk in range(8):
                            nc.tensor.matmul(
                                ps[:, 0:ncols],
                                w1_sb[m // 2][:, kk, (m % 2) * 128:(m % 2 + 1) * 128],
                                xt[:, kk, :],
                                start=(kk == 0),
                                stop=(kk == 7),
                            )
                        nc.scalar.activation(
                            ht[:, m, lo:lo + ncols],
                            ps[:, 0:ncols],
                            AF.Relu,
                            bias=b1_sb[:, m:m + 1],
                        )
                for ch in range(BCH):
                    nch = blk * BCH + ch
                    seg_i = (ch * 128) // segs[0][2]
                    xpc_t = segs[seg_i][0]
                    xc0 = ch * 128 - segs[seg_i][1]
                    # gate2: g[n, q] = sigmoid(h @ w2 + b2), hT stationary
                    g = g_pool.tile([128, C], F16, name="g", tag="g")
                    for half in range(2):
                        ps = g2ps_pool.tile([128, 512], F32, name="g2ps", tag="g2ps")
                        for kk in range(8):
                            nc.tensor.matmul(
                                ps,
                                ht[:, kk, ch * 128:(ch + 1) * 128],
                                w2_sb[:, kk, half * 512:(half + 1) * 512],
                                start=(kk == 0),
                                stop=(kk == 7 and not with_bias),
                            )
                        if with_bias:
                            nc.tensor.matmul(
                                ps,
                                ones_r,
                                b2_r[:, half * 512:(half + 1) * 512],
                                start=False,
                                stop=True,
                            )
                        nc.scalar.activation(
                            g[:, half * 512:(half + 1) * 512], ps, AF.Sigmoid
                        )
                    # kv projection: kv[n, m], xpT stationary
                    k_sb = k_pool.tile([128, C], F16, name="k_sb", tag="k_sb")
                    vg = vg_pool.tile([128, C], F16, name="vg", tag="vg")
                    for q in range(4):
                        ps = kvps_pool.tile([128, 512], F32, name="kvps", tag="kvps")
                        for kk in range(8):
                            nc.tensor.matmul(
                                ps,
                                xpc_t[:, kk, xc0:xc0 + 128],
                                wkv_sb[:, kk, q * 512:(q + 1) * 512],
                                start=(kk == 0),
                                stop=(kk == 7),
                            )
                        if q < 2:
                            # last chunk: keep DVE free so softmax starts sooner
                            if nch == N // 128 - 1:
                                nc.scalar.copy(k_sb[:, q * 512:(q + 1) * 512], ps)
                            else:
                                nc.vector.tensor_copy(k_sb[:, q * 512:(q + 1) * 512], ps)
                        else:
                            qq = q - 2
                            nc.vector.tensor_mul(
                                vg[:, qq * 512:(qq + 1) * 512],
                                ps,
                                g[:, qq * 512:(qq + 1) * 512],
                            )
                    emit_ctx()
                    pending.append((k_sb, vg, nch))
                if blk == 0:
                    xpt0_stack.close()
            emit_ctx()

        # =========================================================
        # Softmax over d (free dim of ctx^T) + build block-diag S pairs
        # =========================================================
        with ExitStack() as sm:
            smp = sm.enter_context(tc.tile_pool(name="smpool", bufs=1))
            smps = sm.enter_context(tc.tile_pool(name="smps", bufs=2, space="PSUM"))
            dmy_pool = sm.enter_context(
                tc.tile_pool(name="dmy", bufs=2, space="PSUM")
            )

            def pe_keepwarm(nmm):
                # dependency-free matmuls on resident weights: keep the PE
                # p-state ramped through the softmax serial chain (results
                # are discarded).
                for _ in range(nmm):
                    dps = dmy_pool.tile([128, 512], F32, name="dmy", tag="dmy")
                    nc.tensor.matmul(
                        dps, w2_sb[:, 0, 0:128], w2_sb[:, 0, 0:512],
                        start=True, stop=True,
                    )

            pe_keepwarm(4)
            # softmax chain pipelined in four 256-col quarters across DVE/ScalarE
            maxs = smp.tile([128, 16], F32, name="maxs")
            cmx = smp.tile([128, 1024], F32, name="cmx")
            et = smp.tile([128, 1024], F32, name="et")
            sums = smp.tile([128, 16], F32, name="sums")
            recs = smp.tile([128, 16], F32, name="recs")
            stb = smp.tile([128, 1024], F16, name="stb")
            for h in range(4):
                sl = slice(h * 256, (h + 1) * 256)
                gsl = slice(h * 4, (h + 1) * 4)
                nc.vector.tensor_reduce(
                    maxs[:, gsl],
                    ctx_acc[:, sl].rearrange("p (g d) -> p g d", g=4),
                    axis=mybir.AxisListType.X,
                    op=mybir.AluOpType.max,
                )
                nc.vector.tensor_sub(
                    cmx[:, sl].rearrange("p (g d) -> p g d", g=4),
                    ctx_acc[:, sl].rearrange("p (g d) -> p g d", g=4),
                    maxs[:, gsl].unsqueeze(-1).broadcast_to([128, 4, 64]),
                )
                nc.scalar.activation(et[:, sl], cmx[:, sl], AF.Exp,
                                     scale=float(SCALE))
                nc.vector.tensor_reduce(
                    sums[:, gsl],
                    et[:, sl].rearrange("p (g d) -> p g d", g=4),
                    axis=mybir.AxisListType.X,
                    op=mybir.AluOpType.add,
                )
                nc.vector.reciprocal(recs[:, gsl], sums[:, gsl])
                nc.vector.tensor_mul(
                    stb[:, sl].rearrange("p (g d) -> p g d", g=4),
                    et[:, sl].rearrange("p (g d) -> p g d", g=4),
                    recs[:, gsl].unsqueeze(-1).broadcast_to([128, 4, 64]),
                )
                # stb[:, j*128:(j+1)*128] = softmaxed ctxT pair [e(2), d(2)];
                # transpose -> [d(2), e(2)], mask off off-diagonal garbage.
                for j in range(2 * h, 2 * h + 2):
                    tp = smps.tile([128, 128], F16, name="smtp", tag="smtp")
                    nc.tensor.transpose(
                        tp, stb[:, j * 128:(j + 1) * 128], identh_sb
                    )
                    nc.vector.tensor_mul(spairs[j], tp, maskh_sb)
                if h < 3:
                    pe_keepwarm(2)

        # =========================================================
        # Phase B (transposed out): oT[j*128:(j+1)*128, nblk] =
        #   spair_j^T @ xqT[j-pair rows, nblk].  spair_j stays stationary
        #   across all n (8 LDWEIGHTS total), xqT streams 1024 cols/MM.
        # =========================================================
        with ExitStack() as pb:
            oo_pool = pb.enter_context(tc.tile_pool(name="oo", bufs=3))
            bops_pool = pb.enter_context(
                tc.tile_pool(name="bops", bufs=8, space="PSUM")
            )
            for j in range(8):
                # one oT row-block [128, N] per head pair, two half DMAs out
                oo = oo_pool.tile([128, N], F16, name="oo", tag="oo")
                for blk in range(NBLK):
                    xqt = xqt_tiles[blk]
                    for h in range(2):
                        ops = bops_pool.tile([128, 512], F32, name="ops",
                                             tag="ops")
                        nc.tensor.matmul(
                            ops,
                            spairs[j],
                            xqt[:, j, h * 512:(h + 1) * 512],
                            start=True,
                            stop=True,
                        )
                        lo = blk * BN + h * 512
                        if (blk * 2 + h) % 2 == 0:
                            nc.vector.tensor_copy(oo[:, lo:lo + 512], ops)
                        else:
                            nc.scalar.copy(oo[:, lo:lo + 512], ops)
                    if j == 7:
                        # finer final DMAs shorten the end-of-kernel flush
                        nc.sync.dma_start(
                            o[j * 128:(j + 1) * 128, blk * BN:(blk + 1) * BN],
                            oo[:, blk * BN:(blk + 1) * BN],
                        )
                    elif blk % 2 == 1:
                        nc.sync.dma_start(
                            o[j * 128:(j + 1) * 128,
                              (blk - 1) * BN:(blk + 1) * BN],
                            oo[:, (blk - 1) * BN:(blk + 1) * BN],
                        )

    nc.compile()
    return nc


def _get_program(with_bias=False):
    key = ("nc", bool(with_bias))
    if key not in _CACHE:
        _CACHE[key] = _build_program(with_bias)
    return _CACHE[key]


def make_in_maps(x1, x2, Wkv1, Wkv2, g1_w1, g1_b1, g1_w2, g1_b2,
                 g2_w1, g2_b1, g2_w2, g2_b2):
    """Core (s, b): cores 0-3 = (s=0, b), cores 4-7 = (s=1, b)."""
    f16 = np.float16
    ident = np.eye(128, dtype=f16)
    mask = np.zeros((128, 128), dtype=f16)
    mask[:64, :64] = np.float16(1.0)
    mask[64:, 64:] = np.float16(1.0)
    asc = np.ascontiguousarray

    def ctile(a2d, nseg, ncols):
        # [C, W] -> SBUF layout segments [nseg, 128p, 8k, ncols]
        t = a2d.reshape(8, 128, a2d.shape[1]).transpose(1, 0, 2)  # [p, k, W]
        return asc(t.reshape(128, 8, nseg, ncols).transpose(2, 0, 1, 3))

    def xforms(x):
        # per batch: transposed fp16, pre-tiled as blk-0 quarters, halves 1-3,
        # and full xq blocks
        xT = np.asarray(x, np.float32).T.astype(f16)  # [C, N]
        base = xT.reshape(8, 128, N).transpose(1, 0, 2)  # [p, k, n]
        xq_b = asc(base.reshape(128, 8, 4, BN).transpose(2, 0, 1, 3))
        xp_q = asc(base[:, :, 0:BN].reshape(128, 8, 4, 256).transpose(2, 0, 1, 3))
        xp_h = asc(base[:, :, BN:].reshape(128, 8, 6, 512).transpose(2, 0, 1, 3))
        return xp_q, xp_h, xq_b

    x1f = [xforms(x1[b]) for b in range(x1.shape[0])]
    x2f = [xforms(x2[b]) for b in range(x2.shape[0])]
    wkv1p = ctile(np.asarray(Wkv1, np.float32).astype(f16), 1, 2048)[0]
    wkv2p = ctile(np.asarray(Wkv2, np.float32).astype(f16), 1, 2048)[0]
    w11q = ctile(np.asarray(g1_w1, np.float32).astype(f16), 4, 256)
    w21q = ctile(np.asarray(g2_w1, np.float32).astype(f16), 4, 256)
    w12p = ctile(np.asarray(g1_w2, np.float32).astype(f16), 1, 1024)[0]
    w22p = ctile(np.asarray(g2_w2, np.float32).astype(f16), 1, 1024)[0]
    b11 = np.asarray(g1_b1, np.float32)
    b21 = np.asarray(g2_b1, np.float32)
    b12h = np.asarray(g1_b2, np.float32).astype(f16)
    b22h = np.asarray(g2_b2, np.float32).astype(f16)
    in_maps = []
    for core in range(8):
        s, b = core // 4, core % 4
        if s == 0:
            xpf, xqf = x1f[b], x2f[b]
            m = dict(wkvp=wkv1p, b1=b11, w2p=w12p, b2=b12h)
            w1s = w11q
        else:
            xpf, xqf = x2f[b], x1f[b]
            m = dict(wkvp=wkv2p, b1=b21, w2p=w22p, b2=b22h)
            w1s = w21q
        for i in range(4):
            m[f"xpq{i}"] = np.ascontiguousarray(xpf[0][i])
            m[f"xqb{i}"] = np.ascontiguousarray(xqf[2][i])
            m[f"w1q{i}"] = np.ascontiguousarray(w1s[i])
        for i in range(6):
            m[f"xph{i}"] = np.ascontiguousarray(xpf[1][i])
        m["identh"] = ident
        m["maskh"] = mask
        in_maps.append(m)
    return in_maps


def kernel(x1, x2, Wkv1, Wkv2, g1_w1, g1_b1, g1_w2, g1_b2,
           g2_w1, g2_b1, g2_w2, g2_b2, _runner=None):
    """Full-input entry point.  Returns (o1, o2), each [4, 4096, 1024] f32."""
    from concourse.bass_utils import run_bass_kernel_spmd

    args = [np.asarray(a, dtype=np.float32) for a in
            (x1, x2, Wkv1, Wkv2, g1_w1, g1_b1, g1_w2, g1_b2,
             g2_w1, g2_b1, g2_w2, g2_b2)]
    with_bias = bool(np.any(args[7]) or np.any(args[11]))  # g1_b2, g2_b2
    nc = _get_program(with_bias)
    in_maps = make_in_maps(*args)
    if _runner is None:
        res = run_bass_kernel_spmd(nc, in_maps, core_ids=list(range(8)))
        results = res.results
    else:
        results = _runner(nc, in_maps)

    B = x1.shape[0]
    o1 = np.empty((B, N, C), dtype=np.float32)
    o2 = np.empty((B, N, C), dtype=np.float32)
    for core in range(8):
        s, b = core // 4, core % 4
        out = np.asarray(results[core]["o"]).T.astype(np.float32)  # [C,N] -> [N,C]
        if s == 0:
            o2[b] = out   # core projected x1 -> ctx1 -> o2 = q2 @ ctx1
        else:
            o1[b] = out
    return (o1, o2)
